# revision 1
# baseline (speedup 1.0000x reference)
"""Causal self-attention (B=4, T=2048, C=1024, H=16) on 8 TRN2 NeuronCores.

Sharding: 8 cores = 4 batches x 2 head-groups (8 heads each). Core c = g*4+b
handles batch b, heads 8g..8g+8 (4 pairs of 2). Inside kernel(): the host
transposes x[b] -> xT [C,T], slices/arranges W_attn columns (Wq pre-scaled by
1/sqrt(D)) and W_proj rows per group, runs one Bass/Tile kernel SPMD on cores
0-7, then sums the two group-partial out^T [C,T] per batch and transposes.

Per-core device pipeline (PE matmuls fp32r; Q/K bf16):
  1. QK^T projection -> Q^T/K^T [128(2 heads), T] per pair
  2. V in natural layout [tk, head, 64] + fused ones column (softmax denom)
  3. per (pair, head), per key-tile jt: scores^T = K_jt @ Q^T over the exact
     causal span (f32 PSUM, 1024-chunks) -> ACT exp -> es (f32r SBUF) ->
     diagonal mask-mul -> PV: [V|1]^T @ es accumulating Y^T+sums [65,T] PSUM
  4. normalize: recip(sums) -> gpsimd partition_broadcast -> DVE mul
  5. output projection: out^T[cout,n] = sum_kp Wp_kp^T @ Y^T_kp -> DMA out
"""
import sys
if '/opt/trn_rl_repo' not in sys.path:
    sys.path.insert(0, '/opt/trn_rl_repo')
import numpy as np
import concourse.bacc as bacc
import concourse.tile as tile
import concourse.mybir as mybir
from concourse import bass_utils

F32 = mybir.dt.float32
F32R = mybir.dt.float32r
BF16 = mybir.dt.bfloat16

N_EMBED = 1024
N_HEAD = 16
D = 64
B_FULL, T_FULL, C_FULL = 4, 2048, 1024
N_GROUPS = 2


def build_kernel(T=T_FULL, C=C_FULL, n_pairs=4, reps=1, n_strip=512, qk_dt=BF16):
    HP = n_pairs * 2
    CIN = HP * D
    n_k = C // 128
    n_jt = T // 128
    n_ts = T // n_strip
    jt_per_strip = n_strip // 128

    nc = bacc.Bacc("TRN2", target_bir_lowering=False, debug=False)
    xt_d = nc.dram_tensor("xt", [C, T], F32R, kind="ExternalInput")
    wqk_d = nc.dram_tensor("wqk", [C, n_pairs * 2 * 128], F32R, kind="ExternalInput")
    wv_d = nc.dram_tensor("wv", [C, n_pairs * 128], F32R, kind="ExternalInput")
    wp_d = nc.dram_tensor("wp", [CIN, C], F32R, kind="ExternalInput")
    mask_d = nc.dram_tensor("mask", [128, 128], F32R, kind="ExternalInput")
    outp_d = nc.dram_tensor("outp", [C, T], F32, kind="ExternalOutput")

    xt_r = xt_d.ap().rearrange("(k p) t -> p k t", p=128)
    wqk_r = wqk_d.ap().rearrange("(k p) m -> p k m", p=128)
    wv_r = wv_d.ap().rearrange("(k p) m -> p k m", p=128)
    wp_r = wp_d.ap().rearrange("(k p) m -> p k m", p=128)

    with tile.TileContext(nc) as tc:
        with tc.tile_pool(name="bigp", bufs=1) as bigp, \
             tc.tile_pool(name="wvp", bufs=1) as wvp, \
             tc.tile_pool(name="xwp", bufs=2) as xwp, \
             tc.tile_pool(name="qkp", bufs=1) as qkp, \
             tc.tile_pool(name="vp", bufs=1) as vp, \
             tc.tile_pool(name="maskp", bufs=1) as maskp, \
             tc.tile_pool(name="esp", bufs=2) as esp, \
             tc.tile_pool(name="normp", bufs=1) as normp, \
             tc.tile_pool(name="osbp", bufs=2) as osbp, \
             tc.tile_pool(name="ps_a", bufs=2, space="PSUM") as ps_a, \
             tc.tile_pool(name="ps_y", bufs=1, space="PSUM") as ps_y:

            def body(_i=None, unroll=1):
                mask_sb = maskp.tile([128, 128], F32R)
                nc.sync.dma_start(out=mask_sb[:], in_=mask_d.ap())
                wqk_sb = bigp.tile([128, n_k, n_pairs * 2 * 128], F32R, tag="big")
                nc.sync.dma_start(out=wqk_sb[:], in_=wqk_r)
                wv_sb = wvp.tile([128, n_k, n_pairs * 128], F32R)
                nc.sync.dma_start(out=wv_sb[:], in_=wv_r)

                qt = [qkp.tile([128, T], qk_dt, tag=f"qt{p}", name=f"qt{p}")
                      for p in range(n_pairs)]
                kt = [qkp.tile([128, T], qk_dt, tag=f"kt{p}", name=f"kt{p}")
                      for p in range(n_pairs)]
                v_aug = vp.tile([128, n_jt, HP, 65], F32R)
                nc.vector.memset(v_aug[:].bitcast(F32), 1.0)

                # ---- phase 1: projections, streamed over token strips ----
                for s in range(n_ts):
                    sl = slice(s * n_strip, (s + 1) * n_strip)
                    xs = xwp.tile([128, n_k, n_strip], F32R, tag="xw")
                    nc.sync.dma_start(out=xs[:], in_=xt_r[:, :, sl])
                    for p in range(n_pairs):
                        for qk in range(2):
                            ps = ps_a.tile([128, n_strip], F32, tag="a")
                            for k in range(n_k):
                                nc.tensor.matmul(
                                    ps[:],
                                    wqk_sb[:, k, (p * 2 + qk) * 128:(p * 2 + qk + 1) * 128],
                                    xs[:, k, :],
                                    start=(k == 0), stop=(k == n_k - 1))
                            dst = (qt if qk == 0 else kt)[p]
                            nc.any.tensor_copy(dst[:, sl], ps[:])
                    for nt in range(jt_per_strip):
                        psv = ps_y.tile([128, n_pairs * 128], F32, tag="y")
                        for k in range(n_k):
                            nc.tensor.matmul(
                                psv[:], xs[:, k, nt * 128:(nt + 1) * 128],
                                wv_sb[:, k, :],
                                start=(k == 0), stop=(k == n_k - 1))
                        jt = s * jt_per_strip + nt
                        nc.any.tensor_copy(
                            v_aug[:, jt, :, 0:64],
                            psv[:].rearrange("q (h d) -> q h d", d=D))

                # ---- phase 2: attention per (pair, head) ----
                ysb = bigp.tile([128, n_pairs, T], F32R, tag="big")
                for p in range(n_pairs):
                    for h in range(2):
                        hh = p * 2 + h
                        hs = slice(h * 64, (h + 1) * 64)
                        y_ps = ps_y.tile([65, T], F32, tag="y")
                        CH = min(2 * n_strip, T)
                        for jt in range(n_jt):
                            s0 = jt // jt_per_strip
                            off = 128 * jt - n_strip * s0
                            lo = 128 * jt
                            es = esp.tile([128, T], F32R, tag="es")
                            # per chunk: scores -> exp -> PV (pipelined)
                            for c in range(lo // CH, T // CH):
                                cw_lo = max(lo, c * CH)
                                scores = ps_a.tile([128, CH], F32, tag="a")
                                s_first = cw_lo // n_strip
                                for s in range(s_first, (c + 1) * CH // n_strip):
                                    a = max(cw_lo, s * n_strip)
                                    n = (s + 1) * n_strip - a
                                    if n < 256 and (s + 1) * n_strip - c * CH >= 256:
                                        a = (s + 1) * n_strip - 256
                                        n = 256
                                    nc.tensor.matmul(
                                        scores[:, a - c * CH:a - c * CH + n],
                                        kt[p][hs, lo:lo + 128],
                                        qt[p][hs, a:a + n],
                                        start=True, stop=True)
                                nc.scalar.activation(
                                    out=es[:, cw_lo:(c + 1) * CH],
                                    in_=scores[:, cw_lo - c * CH:CH],
                                    func=mybir.ActivationFunctionType.Exp)
                                # PV strips of this chunk; masked diagonal strip
                                # last so unmasked strips don't wait on the mask.
                                full_start = (s0 + 1 if cw_lo == lo
                                              else (c * CH) // n_strip)
                                for s in range(full_start, (c + 1) * CH // n_strip):
                                    nc.tensor.matmul(
                                        y_ps[:, s * n_strip:(s + 1) * n_strip],
                                        v_aug[:, jt, hh, :],
                                        es[:, s * n_strip:(s + 1) * n_strip],
                                        start=(jt == 0),
                                        stop=(jt == (s + 1) * jt_per_strip - 1),
                                        skip_group_check=True)
                                if cw_lo == lo:
                                    nc.vector.tensor_mul(
                                        es[:, lo:lo + 128], es[:, lo:lo + 128],
                                        mask_sb[:])
                                    pv_n = min(n_strip - off, T - lo)
                                    nc.tensor.matmul(
                                        y_ps[:, lo:lo + pv_n],
                                        v_aug[:, jt, hh, :], es[:, lo:lo + pv_n],
                                        start=(jt == 0),
                                        stop=(jt == (s0 + 1) * jt_per_strip - 1),
                                        skip_group_check=True)
                        # normalize
                        recip = normp.tile([1, T], F32, tag="recip")
                        nc.vector.reciprocal(recip[:], y_ps[64:65, :])
                        bcast = normp.tile([64, T], F32, tag="bcast")
                        nc.gpsimd.partition_broadcast(bcast[:], recip[:])
                        nc.vector.tensor_mul(
                            ysb[h * 64:(h + 1) * 64, p, :], y_ps[0:64, :], bcast[:])

                # ---- phase 3: output projection ----
                wp_sb = xwp.tile([128, CIN // 128, C], F32R, tag="xw")
                nc.sync.dma_start(out=wp_sb[:], in_=wp_r)
                for m in range(C // 128):
                    for s in range(n_ts):
                        sl = slice(s * n_strip, (s + 1) * n_strip)
                        pso = ps_a.tile([128, n_strip], F32, tag="a")
                        for kp in range(CIN // 128):
                            nc.tensor.matmul(
                                pso[:],
                                wp_sb[:, kp, m * 128:(m + 1) * 128],
                                ysb[:, kp, sl],
                                start=(kp == 0), stop=(kp == CIN // 128 - 1))
                        osb = osbp.tile([128, n_strip], F32, tag="osb")
                        nc.any.tensor_copy(osb[:], pso[:])
                        nc.sync.dma_start(
                            out=outp_d.ap()[m * 128:(m + 1) * 128, sl], in_=osb[:])

            if reps == 1:
                body()
            else:
                with tc.For_i(0, reps, 1) as i:
                    body(i)
    nc.compile()
    return nc


def host_inputs(x, W_attn, W_proj, n_groups=N_GROUPS):
    """Per-core input maps. Core order: g * B + b."""
    B, T, C = x.shape
    hp = N_HEAD // n_groups
    n_pairs = hp // 2
    scale = np.float32(1.0 / np.sqrt(D))
    mask = (np.arange(128)[None, :] >= np.arange(128)[:, None]).astype(np.float32)
    in_maps = []
    for g in range(n_groups):
        qk_cols, v_cols = [], []
        for p in range(n_pairs):
            h0 = g * hp + 2 * p
            h1 = h0 + 1
            qk_cols.append(W_attn[:, h0 * D:(h0 + 1) * D] * scale)
            qk_cols.append(W_attn[:, h1 * D:(h1 + 1) * D] * scale)
            qk_cols.append(W_attn[:, C + h0 * D:C + (h0 + 1) * D])
            qk_cols.append(W_attn[:, C + h1 * D:C + (h1 + 1) * D])
            v_cols.append(W_attn[:, 2 * C + h0 * D:2 * C + (h0 + 1) * D])
            v_cols.append(W_attn[:, 2 * C + h1 * D:2 * C + (h1 + 1) * D])
        wqk = np.ascontiguousarray(np.concatenate(qk_cols, axis=1), dtype=np.float32)
        wv = np.ascontiguousarray(np.concatenate(v_cols, axis=1), dtype=np.float32)
        wp = np.ascontiguousarray(W_proj[g * hp * D:(g + 1) * hp * D], dtype=np.float32)
        for b in range(B):
            xt = np.ascontiguousarray(x[b].T, dtype=np.float32)
            in_maps.append({"xt": xt, "wqk": wqk, "wv": wv, "wp": wp, "mask": mask})
    return in_maps


def host_gather(results, B, T, C, n_groups=N_GROUPS):
    out = np.zeros((B, T, C), dtype=np.float32)
    for g in range(n_groups):
        for b in range(B):
            out[b] += results[g * B + b]["outp"].T
    return out


_NC_CACHE = {}


def kernel(x, W_attn, W_proj):
    x = np.asarray(x, dtype=np.float32)
    W_attn = np.asarray(W_attn, dtype=np.float32)
    W_proj = np.asarray(W_proj, dtype=np.float32)
    B, T, C = x.shape
    if "nc" not in _NC_CACHE:
        _NC_CACHE["nc"] = build_kernel(T=T, C=C)
    nc = _NC_CACHE["nc"]
    in_maps = host_inputs(x, W_attn, W_proj)
    res = bass_utils.run_bass_kernel_spmd(nc, in_maps, core_ids=list(range(8)))
    return host_gather(res.results, B, T, C)



# revision 8
# speedup vs baseline: 1.4800x; 1.4800x over previous
"""Causal self-attention (B=4, T=2048, C=1024, H=16) on 8 TRN2 NeuronCores.

Sharding: 8 cores = 4 batches x 2 head-groups (8 heads each). Core c = g*4+b
handles batch b, heads 8g..8g+8 (4 pairs of 2). Host transposes x[b] -> xT
[C,T] in bf16, slices W_attn columns (Wq pre-scaled by 1/sqrt(D)) and W_proj
rows per group (bf16), runs one Bass/Tile kernel SPMD on cores 0-7, then sums
the two group-partial out^T [C,T] (bf16) per batch and transposes to f32.

Per-core device pipeline (all matmul inputs bf16, PSUM f32):
  A. projections, weight-stationary: QK^T -> qt/kt [128(2 heads), T] bf16 per
     pair (PSUM [128,T] acc over k, evac on DVE/ACT); V in natural layout
     v_aug [tok, head, 64|1] interleaved between QK groups.
  B. attention per (pair, head), software-pipelined over key-tiles jt:
     scores^T = K_jt @ Q^T over causal span (1024-chunks, <=512/bank segs)
     -> ACT exp -> es bf16; DVE mask-mul on the 128-wide diagonal; PV
     [V|1]^T @ es accumulates y^T+sums [65,T] PSUM, trailing one jt so ACT
     (the phase bottleneck) never stalls. Then DVE evac y->SBUF, DVE recip,
     Pool partition_broadcast, DVE normalize-mul -> ysb bf16.
  C. output projection, weight-stationary over T-halves (PSUM "a" slots),
     copies alternate DVE/ACT, DMA out bf16.
"""
import sys
if '/opt/trn_rl_repo' not in sys.path:
    sys.path.insert(0, '/opt/trn_rl_repo')
import numpy as np
import ml_dtypes
import concourse.bacc as bacc
import concourse.tile as tile
import concourse.mybir as mybir
from concourse import bass_utils

F32 = mybir.dt.float32
BF16 = mybir.dt.bfloat16
EXP = mybir.ActivationFunctionType.Exp

N_EMBED = 1024
N_HEAD = 16
D = 64
B_FULL, T_FULL, C_FULL = 4, 2048, 1024
N_GROUPS = 2


def build_kernel(T=T_FULL, C=C_FULL, n_pairs=4, reps=1, out_dt=BF16):
    HP = n_pairs * 2          # heads per core
    CIN = HP * D              # 512
    n_k = C // 128            # contraction tiles
    n_jt = T // 128           # key tiles
    n_s = T // 512            # psum-bank strips
    CH = 1024                 # scores chunk (2 psum banks)

    nc = bacc.Bacc("TRN2", target_bir_lowering=False, debug=False)
    xt_d = nc.dram_tensor("xt", [C, T], BF16, kind="ExternalInput")
    wqk_d = nc.dram_tensor("wqk", [C, n_pairs * 2 * 128], BF16, kind="ExternalInput")
    wv_d = nc.dram_tensor("wv", [C, n_pairs * 128], BF16, kind="ExternalInput")
    wp_d = nc.dram_tensor("wp", [CIN, C], BF16, kind="ExternalInput")
    mask_d = nc.dram_tensor("mask", [128, 128], BF16, kind="ExternalInput")
    outp_d = nc.dram_tensor("outp", [C, T], out_dt, kind="ExternalOutput")

    xt_r = xt_d.ap().rearrange("(k p) t -> p k t", p=128)
    wqk_r = wqk_d.ap().rearrange("(k p) m -> p k m", p=128)
    wv_r = wv_d.ap().rearrange("(k p) m -> p k m", p=128)
    wp_r = wp_d.ap().rearrange("(k p) m -> p k m", p=128)

    with tile.TileContext(nc) as tc:
        with tc.tile_pool(name="wts", bufs=1) as wts, \
             tc.tile_pool(name="xsp", bufs=1) as xsp, \
             tc.tile_pool(name="qkp", bufs=1) as qkp, \
             tc.tile_pool(name="vp", bufs=1) as vp, \
             tc.tile_pool(name="esp", bufs=2) as esp, \
             tc.tile_pool(name="normp", bufs=2) as normp, \
             tc.tile_pool(name="ysbp", bufs=1) as ysbp, \
             tc.tile_pool(name="osbp", bufs=2) as osbp, \
             tc.tile_pool(name="ps_big", bufs=1, space="PSUM") as ps_big, \
             tc.tile_pool(name="ps_a", bufs=2, space="PSUM") as ps_a:

            def body(_i=None):
                # ---------------- DMAs (SP queue: weights; ACT queue: x) ---
                wqk_sb = wts.tile([128, n_k, n_pairs * 2 * 128], BF16, tag="wqk")
                nc.sync.dma_start(out=wqk_sb[:], in_=wqk_r)
                wv_sb = wts.tile([128, n_k, n_pairs * 128], BF16, tag="wv")
                nc.sync.dma_start(out=wv_sb[:], in_=wv_r)
                mask_sb = wts.tile([128, 128], BF16, tag="mask")
                nc.sync.dma_start(out=mask_sb[:], in_=mask_d.ap())
                wp_sb = wts.tile([128, CIN // 128, C], BF16, tag="wp")
                nc.sync.dma_start(out=wp_sb[:], in_=wp_r)
                xs = xsp.tile([128, n_k, T], BF16, tag="xs")
                for s in range(n_s):
                    sl = slice(s * 512, (s + 1) * 512)
                    nc.scalar.dma_start(out=xs[:, :, sl], in_=xt_r[:, :, sl])

                qt = [qkp.tile([128, T], BF16, tag=f"qt{p}", name=f"qt{p}")
                      for p in range(n_pairs)]
                kt = [qkp.tile([128, T], BF16, tag=f"kt{p}", name=f"kt{p}")
                      for p in range(n_pairs)]
                v_aug = vp.tile([128, n_jt, HP, 65], BF16)
                nc.vector.memset(v_aug[:, :, :, 64:65], 1.0)

                # ---------------- phase A: projections ---------------------
                # (p,qk)-outer weight-stationary QK; two V token-tiles
                # interleaved per group to hide the PSUM evacuation.
                def v_tile(nt, copy_fn):
                    psv = ps_a.tile([128, n_pairs * 128], F32, tag="a")
                    for k in range(n_k):
                        nc.tensor.matmul(
                            psv[:], xs[:, k, nt * 128:(nt + 1) * 128],
                            wv_sb[:, k, :], start=(k == 0), stop=(k == n_k - 1))
                    copy_fn(
                        v_aug[:, nt, :, 0:64],
                        psv[:].rearrange("q (h d) -> q h d", d=D))

                for i in range(2 * n_pairs):
                    p, qk = i // 2, i % 2
                    acc = ps_big.tile([128, T], F32, tag="big")
                    for k in range(n_k):
                        for s in range(n_s):
                            nc.tensor.matmul(
                                acc[:, s * 512:(s + 1) * 512],
                                wqk_sb[:, k, i * 128:(i + 1) * 128],
                                xs[:, k, s * 512:(s + 1) * 512],
                                start=(k == 0), stop=(k == n_k - 1),
                                skip_group_check=True)
                    dst = (qt if qk == 0 else kt)[p]
                    (nc.vector.tensor_copy if i % 2 == 0 else nc.scalar.copy)(
                        dst[:], acc[:])
                    v_tile(2 * i, nc.vector.tensor_copy)
                    v_tile(2 * i + 1, nc.scalar.copy)

                # ---------------- phase B: attention ------------------------
                ysb = ysbp.tile([128, n_pairs, T], BF16)
                for p in range(n_pairs):
                    for h in range(2):
                        hh = p * 2 + h
                        hs = slice(h * 64, (h + 1) * 64)
                        y_ps = ps_big.tile([65, T], F32, tag="big")

                        y_sb = normp.tile([65, T], F32, tag="ysb")

                        def emit_pv(jt, es):
                            lo = 128 * jt
                            s0 = lo // 512
                            d_end = (s0 + 1) * 512
                            nc.tensor.matmul(
                                y_ps[:, lo:d_end], v_aug[:, jt, hh, :],
                                es[:, lo:d_end], start=(jt == 0),
                                stop=(jt == (s0 + 1) * 4 - 1),
                                skip_group_check=True)
                            for s in range(s0 + 1, n_s):
                                nc.tensor.matmul(
                                    y_ps[:, s * 512:(s + 1) * 512],
                                    v_aug[:, jt, hh, :],
                                    es[:, s * 512:(s + 1) * 512],
                                    start=(jt == 0), stop=False,
                                    skip_group_check=True)
                            if jt % 4 == 3:
                                # strip s0 is complete: evacuate it now so the
                                # next head's PV never waits on the PSUM slot
                                ssl = slice(s0 * 512, d_end)
                                nc.vector.tensor_copy(
                                    y_sb[:, ssl], y_ps[:, ssl])

                        prev = None
                        for jt in range(n_jt):
                            lo = 128 * jt
                            es = esp.tile([128, T], BF16, tag="es")
                            for c in range(lo // CH, T // CH):
                                cw_lo = max(lo, c * CH)
                                ps = ps_a.tile([128, CH], F32, tag="a")
                                a = cw_lo
                                while a < (c + 1) * CH:
                                    e = min((a // 512 + 1) * 512, (c + 1) * CH)
                                    nc.tensor.matmul(
                                        ps[:, a - c * CH:e - c * CH],
                                        kt[p][hs, lo:lo + 128],
                                        qt[p][hs, a:e],
                                        start=True, stop=True)
                                    a = e
                                nc.scalar.activation(
                                    out=es[:, cw_lo:(c + 1) * CH],
                                    in_=ps[:, cw_lo - c * CH:CH], func=EXP)
                            nc.vector.tensor_mul(
                                es[:, lo:lo + 128], es[:, lo:lo + 128], mask_sb[:])
                            if prev is not None:
                                emit_pv(*prev)
                            prev = (jt, es)
                        emit_pv(*prev)

                        # normalize in SBUF, per T-half to shorten the tail
                        recip = normp.tile([1, T], F32, tag="recip")
                        bcast = normp.tile([64, T], F32, tag="bcast")
                        for u in range(2):
                            usl = slice(u * (T // 2), (u + 1) * (T // 2))
                            nc.vector.reciprocal(recip[:, usl], y_sb[64:65, usl])
                            nc.gpsimd.partition_broadcast(
                                bcast[:, usl], recip[:, usl])
                            nc.vector.tensor_mul(
                                ysb[h * 64:(h + 1) * 64, p, usl],
                                y_sb[0:64, usl], bcast[:, usl])

                # ---------------- phase C: output projection ----------------
                for half in range(2):
                    sl = slice(half * 1024, (half + 1) * 1024)
                    for m in range(C // 128):
                        pso = ps_a.tile([128, CH], F32, tag="a")
                        for kp in range(CIN // 128):
                            for s in range(2):
                                nc.tensor.matmul(
                                    pso[:, s * 512:(s + 1) * 512],
                                    wp_sb[:, kp, m * 128:(m + 1) * 128],
                                    ysb[:, kp, half * 1024 + s * 512:
                                        half * 1024 + (s + 1) * 512],
                                    start=(kp == 0), stop=(kp == CIN // 128 - 1),
                                    skip_group_check=True)
                        osb = osbp.tile([128, CH], out_dt, tag="osb")
                        (nc.vector.tensor_copy if m % 2 == 0 else nc.scalar.copy)(
                            osb[:], pso[:])
                        nc.sync.dma_start(
                            out=outp_d.ap()[m * 128:(m + 1) * 128, sl], in_=osb[:])

            if reps == 1:
                body()
            else:
                with tc.For_i(0, reps, 1) as i:
                    body(i)
    nc.compile()
    return nc


def host_inputs(x, W_attn, W_proj, n_groups=N_GROUPS):
    """Per-core input maps (bf16). Core order: g * B + b."""
    B, T, C = x.shape
    hp = N_HEAD // n_groups
    n_pairs = hp // 2
    bf16 = ml_dtypes.bfloat16
    scale = np.float32(1.0 / np.sqrt(D))
    mask = (np.arange(128)[None, :] >= np.arange(128)[:, None]).astype(bf16)
    in_maps = []
    for g in range(n_groups):
        qk_cols, v_cols = [], []
        for p in range(n_pairs):
            h0 = g * hp + 2 * p
            h1 = h0 + 1
            qk_cols.append(W_attn[:, h0 * D:(h0 + 1) * D] * scale)
            qk_cols.append(W_attn[:, h1 * D:(h1 + 1) * D] * scale)
            qk_cols.append(W_attn[:, C + h0 * D:C + (h0 + 1) * D])
            qk_cols.append(W_attn[:, C + h1 * D:C + (h1 + 1) * D])
            v_cols.append(W_attn[:, 2 * C + h0 * D:2 * C + (h0 + 1) * D])
            v_cols.append(W_attn[:, 2 * C + h1 * D:2 * C + (h1 + 1) * D])
        wqk = np.ascontiguousarray(
            np.concatenate(qk_cols, axis=1)).astype(bf16)
        wv = np.ascontiguousarray(np.concatenate(v_cols, axis=1)).astype(bf16)
        wp = np.ascontiguousarray(
            W_proj[g * hp * D:(g + 1) * hp * D]).astype(bf16)
        for b in range(B):
            xt = np.ascontiguousarray(x[b].T).astype(bf16)
            in_maps.append({"xt": xt, "wqk": wqk, "wv": wv, "wp": wp,
                            "mask": mask})
    return in_maps


def host_gather(results, B, T, C, n_groups=N_GROUPS):
    out = np.zeros((B, T, C), dtype=np.float32)
    for g in range(n_groups):
        for b in range(B):
            out[b] += results[g * B + b]["outp"].astype(np.float32).T
    return out


_NC_CACHE = {}


def kernel(x, W_attn, W_proj):
    x = np.asarray(x, dtype=np.float32)
    W_attn = np.asarray(W_attn, dtype=np.float32)
    W_proj = np.asarray(W_proj, dtype=np.float32)
    B, T, C = x.shape
    if "nc" not in _NC_CACHE:
        _NC_CACHE["nc"] = build_kernel(T=T, C=C)
    nc = _NC_CACHE["nc"]
    in_maps = host_inputs(x, W_attn, W_proj)
    res = bass_utils.run_bass_kernel_spmd(nc, in_maps, core_ids=list(range(8)))
    return host_gather(res.results, B, T, C)


# revision 24
# speedup vs baseline: 4.6210x; 3.1223x over previous
"""Causal self-attention (B=4, T=2048, C=1024, H=16) on 8 TRN2 NeuronCores.

Sharding: 8 cores = 4 batches x 2 head-groups (8 heads each). Core c = g*4+b
handles batch b, heads 8g..8g+8 (4 pairs of 2). Host transposes x[b] -> xT
[C,T] in bf16, slices W_attn columns (Wq pre-scaled by 1/sqrt(D)) and W_proj
rows per group (bf16), runs one Bass/Tile kernel SPMD on cores 0-7, then sums
the two group-partial out^T [C,T] (bf16) per batch and transposes to f32.

Per-core device pipeline (all matmul inputs bf16, PSUM f32):
  A. QK^T projection for pair 0 (weight-stationary over T/2-half PSUM accs)
     with V-projection tiles (natural layout + fused ones column) interleaved.
  B. attention per (pair, head, T/2 query-half), software-pipelined over key
     tiles jt: scores^T = K_jt @ Q^T (<=512 bank-aligned segs) -> one ACT exp
     per jt -> es bf16 -> DVE mask-mul on the diagonal -> PV [V|1]^T @ es
     accumulating y^T+sums [65,1024] PSUM, trailing one jt so ACT (the phase
     bottleneck) never stalls; completed 512-strips evacuate to SBUF at once.
     Normalize per half: DVE recip, Pool partition_broadcast, DVE mul -> bf16.
     The NEXT pair's QK projection is drip-fed between jt steps as 2-matmul
     quanta so the PE stays busy under the ACT-bound phase.
  C. output projection, weight-stationary, PSUM slots rotated across the
     freed pools; copies alternate DVE/ACT; DMA out bf16 on the SP queue.
"""
import sys
if '/opt/trn_rl_repo' not in sys.path:
    sys.path.insert(0, '/opt/trn_rl_repo')
import collections
import numpy as np
import ml_dtypes
import concourse.bacc as bacc
import concourse.tile as tile
import concourse.mybir as mybir
from concourse import bass_utils

F32 = mybir.dt.float32
BF16 = mybir.dt.bfloat16
EXP = mybir.ActivationFunctionType.Exp

N_EMBED = 1024
N_HEAD = 16
D = 64
B_FULL, T_FULL, C_FULL = 4, 2048, 1024
N_GROUPS = 2


def build_kernel(T=T_FULL, C=C_FULL, n_pairs=4, reps=1, out_dt=BF16):
    HP = n_pairs * 2          # heads per core
    CIN = HP * D              # 512
    n_k = C // 128            # contraction tiles
    n_jt = T // 128           # key tiles
    HT = T // 2               # query half

    nc = bacc.Bacc("TRN2", target_bir_lowering=False, debug=False)
    xt_d = nc.dram_tensor("xt", [C, T], BF16, kind="ExternalInput")
    wqk_d = nc.dram_tensor("wqk", [C, n_pairs * 2 * 128], BF16, kind="ExternalInput")
    wv_d = nc.dram_tensor("wv", [C, n_pairs * 128], BF16, kind="ExternalInput")
    wp_d = nc.dram_tensor("wp", [CIN, C], BF16, kind="ExternalInput")
    mask_d = nc.dram_tensor("mask", [128, 128], F32, kind="ExternalInput")
    outp_d = nc.dram_tensor("outp", [C, T], out_dt, kind="ExternalOutput")

    xt_r = xt_d.ap().rearrange("(k p) t -> p k t", p=128)
    wqk_r = wqk_d.ap().rearrange("(k p) m -> p k m", p=128)
    wv_r = wv_d.ap().rearrange("(k p) m -> p k m", p=128)
    wp_r = wp_d.ap().rearrange("(k p) m -> p k m", p=128)

    with tile.TileContext(nc) as tc:
        with tc.tile_pool(name="wts", bufs=1) as wts, \
             tc.tile_pool(name="xsp", bufs=1) as xsp, \
             tc.tile_pool(name="qkp", bufs=1) as qkp, \
             tc.tile_pool(name="vp", bufs=1) as vp, \
             tc.tile_pool(name="esp", bufs=3) as esp, \
             tc.tile_pool(name="normp", bufs=2) as normp, \
             tc.tile_pool(name="ysbp", bufs=1) as ysbp, \
             tc.tile_pool(name="osbp", bufs=4) as osbp, \
             tc.tile_pool(name="ps_y", bufs=1, space="PSUM") as ps_y, \
             tc.tile_pool(name="ps_a", bufs=2, space="PSUM") as ps_a, \
             tc.tile_pool(name="ps_qk", bufs=1, space="PSUM") as ps_qk:

            def body(_i=None):
                # ------------- DMAs (SP: weights; ACT queue: x) -------------
                # SP queue: pair-0 weights, wv, late x strips, rest of wqk.
                # ACT queue: early x strips. First matmul can start ~2.6us.
                wqk_sb = wts.tile([128, n_k, n_pairs * 2 * 128], BF16, tag="wqk")
                wv_sb = wts.tile([128, n_k, n_pairs * 128], BF16, tag="wv")
                mask_sb = wts.tile([128, 128], F32, tag="mask")
                wp_sb = wts.tile([128, CIN // 128, C], BF16, tag="wp")
                xs = xsp.tile([128, n_k, T], BF16, tag="xs")
                nc.sync.dma_start(out=wqk_sb[:, :, 0:256], in_=wqk_r[:, :, 0:256])
                nc.sync.dma_start(out=wv_sb[:], in_=wv_r)
                for s in range(2):
                    sl = slice(s * 512, (s + 1) * 512)
                    nc.scalar.dma_start(out=xs[:, :, sl], in_=xt_r[:, :, sl])
                for s in range(2, T // 512):
                    sl = slice(s * 512, (s + 1) * 512)
                    nc.sync.dma_start(out=xs[:, :, sl], in_=xt_r[:, :, sl])
                nc.sync.dma_start(out=wqk_sb[:, :, 256:], in_=wqk_r[:, :, 256:])
                nc.sync.dma_start(out=mask_sb[:], in_=mask_d.ap())
                nc.sync.dma_start(out=wp_sb[:], in_=wp_r)

                qt = [qkp.tile([128, T], BF16, tag=f"qt{p}", name=f"qt{p}")
                      for p in range(n_pairs)]
                kt = [qkp.tile([128, T], BF16, tag=f"kt{p}", name=f"kt{p}")
                      for p in range(n_pairs)]
                v_aug = vp.tile([128, n_jt, HP, 65], BF16)
                nc.vector.memset(v_aug[:, :, :, 64:65], 1.0)

                # ------------- projection emitters --------------------------
                def v_tile(nt, copy_fn, pool=None, tag="a"):
                    psv = (pool or ps_a).tile(
                        [128, n_pairs * 128], F32, tag=tag, name=f"psv{nt}")
                    for k in range(n_k):
                        nc.tensor.matmul(
                            psv[:], xs[:, k, nt * 128:(nt + 1) * 128],
                            wv_sb[:, k, :], start=(k == 0), stop=(k == n_k - 1))
                    copy_fn(
                        v_aug[:, nt, :, 0:64],
                        psv[:].rearrange("q (h d) -> q h d", d=D))

                def qk_group(p, qk, half):
                    """Emit one (qk, half) projection group immediately."""
                    for q in qk_group_quanta(p, qk, half):
                        q()

                def qk_group_quanta(p, qk, half):
                    """Yield 2-matmul k-step closures for one (qk,half) group."""
                    i = p * 2 + qk
                    box = {}

                    def step(k, box=box, p=p, qk=qk, half=half, i=i):
                        if k == 0:
                            box["acc"] = ps_qk.tile(
                                [128, HT], F32, tag="qkacc",
                                name=f"qkacc{p}_{qk}_{half}")
                        acc = box["acc"]
                        for s in range(2):
                            col = half * HT + s * 512
                            nc.tensor.matmul(
                                acc[:, s * 512:(s + 1) * 512],
                                wqk_sb[:, k, i * 128:(i + 1) * 128],
                                xs[:, k, col:col + 512],
                                start=(k == 0), stop=(k == n_k - 1),
                                skip_group_check=True)
                        if k == n_k - 1:
                            dst = (qt if qk == 0 else kt)[p]
                            nc.vector.tensor_copy(
                                dst[:, half * HT:(half + 1) * HT], acc[:])

                    for k in range(n_k):
                        yield (lambda k=k: step(k))

                # ------------- phase A: pair-0 QK + first-half V ------------
                vq = iter(range(n_jt // 2))
                for half in (0, 1):
                    for qk in (0, 1):
                        qk_group(0, qk, half)
                        for _ in range(2):
                            nt = next(vq, None)
                            if nt is not None:
                                v_tile(nt, nc.vector.tensor_copy)
                for nt in vq:
                    v_tile(nt, nc.scalar.copy if nt % 2 else nc.vector.tensor_copy)

                # ------------- phase B: attention ---------------------------
                quanta = collections.deque()
                ysb = ysbp.tile([128, n_pairs, T], BF16)
                for p in range(n_pairs):
                    if p == 0:
                        # late-half V tiles drip-feed first (consumed from u=1)
                        quanta.extend(
                            (lambda nt=nt: v_tile(
                                nt, nc.vector.tensor_copy, ps_qk, "qkacc"))
                            for nt in range(n_jt // 2, n_jt))
                    if p + 1 < n_pairs:
                        for qk in (0, 1):
                            for half in (0, 1):
                                quanta.extend(qk_group_quanta(p + 1, qk, half))
                    for h in range(2):
                        hh = p * 2 + h
                        hs = slice(h * 64, (h + 1) * 64)
                        for u in range(2):
                            qlo, qhi = u * HT, (u + 1) * HT
                            y_ps = ps_y.tile([65, HT], F32, tag="y")
                            y_sb = normp.tile([65, HT], F32, tag="ysb")

                            def emit_pv(jt, es, y_ps=y_ps, y_sb=y_sb,
                                        hh=hh, qlo=qlo, qhi=qhi):
                                lo = 128 * jt
                                a0 = max(lo, qlo)
                                pieces = []
                                a = a0
                                while a < qhi:
                                    e = min((a // 512 + 1) * 512, qhi)
                                    pieces.append((a, e))
                                    a = e
                                for (a, e) in pieces:
                                    nc.tensor.matmul(
                                        y_ps[:, a - qlo:e - qlo],
                                        v_aug[:, jt, hh, :],
                                        es[:, a - qlo:e - qlo],
                                        start=(jt == 0),
                                        stop=(jt == (a // 512) * 4 + 3),
                                        skip_group_check=True)
                                if jt % 4 == 3 and lo >= qlo:
                                    # diagonal strip complete: evacuate now
                                    s0 = lo // 512
                                    lsl = slice(s0 * 512 - qlo,
                                                (s0 + 1) * 512 - qlo)
                                    nc.vector.tensor_copy(
                                        y_sb[:, lsl], y_ps[:, lsl])

                            pending = collections.deque()
                            for jt in range(8 * (u + 1)):
                                lo = 128 * jt
                                a0 = max(lo, qlo)
                                es = esp.tile([128, HT], BF16, tag="es")
                                ps = ps_a.tile([128, HT], F32, tag="a")
                                a = a0
                                while a < qhi:
                                    e = min((a // 512 + 1) * 512, qhi)
                                    nc.tensor.matmul(
                                        ps[:, a - qlo:e - qlo],
                                        kt[p][hs, lo:lo + 128],
                                        qt[p][hs, a:e],
                                        start=True, stop=True)
                                    a = e
                                if lo >= qlo:
                                    # additive causal mask (-1e30) on the
                                    # diagonal block, pre-exp: PV then only
                                    # depends on exp, never on the DVE
                                    nc.vector.tensor_add(
                                        ps[:, lo - qlo:lo - qlo + 128],
                                        ps[:, lo - qlo:lo - qlo + 128],
                                        mask_sb[:])
                                nc.scalar.activation(
                                    out=es[:, a0 - qlo:HT],
                                    in_=ps[:, a0 - qlo:HT], func=EXP)
                                pending.append((jt, es))
                                if len(pending) > 2:
                                    emit_pv(*pending.popleft())
                                if quanta:
                                    quanta.popleft()()
                            for item in pending:
                                emit_pv(*item)
                                if quanta:
                                    quanta.popleft()()

                            # normalize this half in SBUF
                            recip = normp.tile([1, HT], F32, tag="recip")
                            nc.vector.reciprocal(recip[:], y_sb[64:65, :])
                            bcast = normp.tile([64, HT], F32, tag="bcast")
                            nc.gpsimd.partition_broadcast(bcast[:], recip[:])
                            nc.vector.tensor_mul(
                                ysb[h * 64:(h + 1) * 64, p, qlo:qhi],
                                y_sb[0:64, :], bcast[:])
                while quanta:
                    quanta.popleft()()

                # ------------- phase C: output projection -------------------
                pso_src = [(ps_a, "a"), (ps_a, "a"), (ps_qk, "qkacc"),
                           (ps_y, "y")]
                for half in range(2):
                    sl = slice(half * HT, (half + 1) * HT)
                    for m in range(C // 128):
                        pool, tag = pso_src[m % 4]
                        pso = pool.tile([128, HT], F32, tag=tag, name=f"pso{m}")
                        for kp in range(CIN // 128):
                            for s in range(2):
                                nc.tensor.matmul(
                                    pso[:, s * 512:(s + 1) * 512],
                                    wp_sb[:, kp, m * 128:(m + 1) * 128],
                                    ysb[:, kp, half * HT + s * 512:
                                        half * HT + (s + 1) * 512],
                                    start=(kp == 0), stop=(kp == CIN // 128 - 1),
                                    skip_group_check=True)
                        osb = osbp.tile([128, HT], out_dt, tag="osb")
                        (nc.vector.tensor_copy if m % 2 == 0 else nc.scalar.copy)(
                            osb[:], pso[:])
                        nc.sync.dma_start(
                            out=outp_d.ap()[m * 128:(m + 1) * 128, sl], in_=osb[:])

            if reps == 1:
                body()
            else:
                with tc.For_i(0, reps, 1) as i:
                    body(i)
    nc.compile()
    return nc


def host_inputs(x, W_attn, W_proj, n_groups=N_GROUPS):
    """Per-core input maps (bf16). Core order: g * B + b."""
    B, T, C = x.shape
    hp = N_HEAD // n_groups
    n_pairs = hp // 2
    bf16 = ml_dtypes.bfloat16
    scale = np.float32(1.0 / np.sqrt(D))
    allowed = np.arange(128)[None, :] >= np.arange(128)[:, None]
    mask = np.where(allowed, 0.0, -1e30).astype(np.float32)
    in_maps = []
    for g in range(n_groups):
        qk_cols, v_cols = [], []
        for p in range(n_pairs):
            h0 = g * hp + 2 * p
            h1 = h0 + 1
            qk_cols.append(W_attn[:, h0 * D:(h0 + 1) * D] * scale)
            qk_cols.append(W_attn[:, h1 * D:(h1 + 1) * D] * scale)
            qk_cols.append(W_attn[:, C + h0 * D:C + (h0 + 1) * D])
            qk_cols.append(W_attn[:, C + h1 * D:C + (h1 + 1) * D])
            v_cols.append(W_attn[:, 2 * C + h0 * D:2 * C + (h0 + 1) * D])
            v_cols.append(W_attn[:, 2 * C + h1 * D:2 * C + (h1 + 1) * D])
        wqk = np.ascontiguousarray(
            np.concatenate(qk_cols, axis=1)).astype(bf16)
        wv = np.ascontiguousarray(np.concatenate(v_cols, axis=1)).astype(bf16)
        wp = np.ascontiguousarray(
            W_proj[g * hp * D:(g + 1) * hp * D]).astype(bf16)
        for b in range(B):
            xt = np.ascontiguousarray(x[b].T).astype(bf16)
            in_maps.append({"xt": xt, "wqk": wqk, "wv": wv, "wp": wp,
                            "mask": mask})
    return in_maps


def host_gather(results, B, T, C, n_groups=N_GROUPS):
    out = np.zeros((B, T, C), dtype=np.float32)
    for g in range(n_groups):
        for b in range(B):
            out[b] += results[g * B + b]["outp"].astype(np.float32).T
    return out


_NC_CACHE = {}


def kernel(x, W_attn, W_proj):
    x = np.asarray(x, dtype=np.float32)
    W_attn = np.asarray(W_attn, dtype=np.float32)
    W_proj = np.asarray(W_proj, dtype=np.float32)
    B, T, C = x.shape
    if "nc" not in _NC_CACHE:
        _NC_CACHE["nc"] = build_kernel(T=T, C=C)
    nc = _NC_CACHE["nc"]
    in_maps = host_inputs(x, W_attn, W_proj)
    res = bass_utils.run_bass_kernel_spmd(nc, in_maps, core_ids=list(range(8)))
    return host_gather(res.results, B, T, C)
